# revision 38
# baseline (speedup 1.0000x reference)
"""Trainium2 Bass kernel for nn_Encoder_82910048682485 (binary-tree GNN encoder).

Structure exploited: in the heap-layout complete binary tree, the children of
the contiguous parent range [2^l-1, 2^(l+1)-1) are exactly the contiguous
range [2^(l+1)-1, 2^(l+2)-1), and parent p's children are cols 2s / 2s+1 of
that block.  So the whole computation is a chain of matmuls over shrinking
contiguous blocks — no real gather/scatter.

Sharding: data-parallel over the 8 subtrees rooted at nodes 7..14 (level 3).
Each core owns 2^15 leaves and computes its subtree's 2^16-1 node embeddings.
The top 7 nodes (levels 0..2) are computed on host (7 rows of a 256->128 MLP,
~0.001% of FLOPs).

On-chip layout is transposed: embeddings are stored [EMB=128 partitions,
nodes as free dim].  Then the even/odd child split needed by the cell MLP is
just a stride-2 free-dim access pattern, and each level-up step is 6 PE
matmuls + 2 leaky-relu passes.  Leaf chunks stream in and a binary-counter
cascade of per-level pending buffers fuses all levels in SBUF (each node
embedding is written to HBM exactly once, read back never).

Matmul operands are fp16 by default (fp32r runs in the PE's half-duty
fp32-HIGH mode and never warms the HAM clock gate; fp16 streams 1 row/cycle
at 2.4 GHz like bf16 but keeps 10 mantissa bits, and halves the output DMA
bytes).  PSUM accumulation stays fp32.
When all biases are zero (true for this model), leaky-relu work is split
between the Scalar engine (native Lrelu) and the idle Vector engine
(0.01*x + 0.99*relu(x), two fused ALU ops), and the two halves of the hidden
layer share one [128, w] PSUM tile so one pass covers both.
"""

import sys

for _p in ("/opt/trn_rl_repo",):
    if _p not in sys.path:
        sys.path.insert(0, _p)

import numpy as np

import concourse.bacc as bacc
import concourse.bass as bass
import concourse.mybir as mybir
from concourse import tile
from concourse.bass_utils import run_bass_kernel_spmd

DEPTH = 18
EMB = 128
HID = 256
VAL = 32
N_LEAVES = 2 ** DEPTH
N_NODES = 2 ** (DEPTH + 1) - 1
N_CORES = 8
SUB = DEPTH - 3              # per-core subtree: levels 0..SUB, 2^SUB leaves
ALPHA = 0.01                 # jax.nn.leaky_relu default negative_slope

F32 = mybir.dt.float32
F32R = mybir.dt.float32r
BF16 = mybir.dt.bfloat16
FP16 = mybir.dt.float16
LRELU = mybir.ActivationFunctionType.Lrelu

# wpack column layout ([128, WPACK_COLS], matmul dtype):
_W1A = 0          # W1[0:128, :]    cols 0:256
_W1B = 256        # W1[128:256, :]  cols 256:512
_W2A = 512        # W2[0:128, :]    cols 512:640
_W2B = 640        # W2[128:256, :]  cols 640:768
_WE = 768         # We (rows 0:32)  cols 768:896
WPACK_COLS = 896
# bias tile columns ([128, 4] fp32): b1[0:128], b1[128:256], b2, be

# matmul operand precision (see module docstring); fp16 streams 1 row/cycle
# like bf16 but keeps 10 mantissa bits (values here are O(1), far from
# fp16 range limits)
MM_DT = FP16

# lowest subtree level computed on device; levels below are finished on the
# host (full-tree levels < SHIP_D + 3, i.e. 32767 of 524287 nodes), trimming
# the strictly serial top-of-tree cascade off the device critical path
SHIP_D = 12


def build_nc(sub=SUB, ch=1024, wcap=1024, n_lv_dmas=16,
             mm_dt=MM_DT, zero_bias=True, dve_out=True, dve_h_every=0,
             f32r_min_n=2, ship_d=0):
    """Build the per-core SPMD Bass program.

    sub:        subtree leaf level (leaves = 2^sub)
    ch:         leaf chunk width (columns per leaf psum tile, <= 1024)
    wcap:       max pending-buffer width (columns consumed per upward step)
    zero_bias:  enables the fused single-pass h activation and DVE routing
                (only correct when b1 == 0)
    dve_out:    route the parent-output leaky-relu to the Vector engine
    dve_h_every: if k > 0, route every k-th h-activation to DVE as well
    ship_d:     lowest subtree level computed on device.  Levels < ship_d
                are left to the host (the tail of the cascade is a strictly
                serial chain of one consume per level, so trimming levels
                off the top trims serial latency 1:1).
    """
    n_leaves = 2 ** sub
    n_out = 2 ** (sub + 1) - 1
    ch = min(ch, n_leaves)
    assert n_leaves % ch == 0
    n_chunks = n_leaves // ch
    assert n_chunks % n_lv_dmas == 0
    qs = n_leaves // n_lv_dmas          # leaf columns per input DMA
    chunks_per_q = n_chunks // n_lv_dmas
    assert 0 <= ship_d < sub

    def width(d):
        return min(wcap, 2 ** d)

    nc = bacc.Bacc("TRN2", target_bir_lowering=False, debug=False)
    lv_d = nc.dram_tensor("lvT", [VAL, n_leaves], mm_dt, kind="ExternalInput").ap()
    wp_d = nc.dram_tensor("wpack", [128, WPACK_COLS], mm_dt, kind="ExternalInput").ap()
    bias_d = nc.dram_tensor("bias", [128, 4], F32, kind="ExternalInput").ap()
    out_d = nc.dram_tensor("outT", [EMB, n_out], mm_dt, kind="ExternalOutput").ap()

    with tile.TileContext(nc) as tc:
        import contextlib
        with contextlib.ExitStack() as ctx:
            const_pool = ctx.enter_context(tc.tile_pool(name="const", bufs=1))
            pend_pool = ctx.enter_context(tc.tile_pool(name="pend", bufs=4))
            hs_pool = ctx.enter_context(tc.tile_pool(name="hs", bufs=4))
            dvetmp_pool = ctx.enter_context(tc.tile_pool(name="dvetmp", bufs=4))
            # PSUM budget (8 banks): leaf [128,1024]x1 = 2, h [128,1024]x2 = 4,
            # o [128,512]x2 = 2.
            ps_leaf = ctx.enter_context(tc.tile_pool(name="psl", bufs=1, space="PSUM"))
            ps_h = ctx.enter_context(tc.tile_pool(name="psh", bufs=2, space="PSUM"))
            ps_o = ctx.enter_context(tc.tile_pool(name="pso", bufs=2, space="PSUM"))

            wp = const_pool.tile([128, WPACK_COLS], mm_dt, tag="wp")
            # We block first (it is all the leaf matmuls need; DMA dispatches
            # serialize ~650ns each on the Sync engine)
            nc.sync.dma_start(wp[:, _WE:], wp_d[:, _WE:])

            # Leaf values live in a manual 3-slot ring padded to 128
            # partitions.  Rows 32:128 are zeroed once (and the We block's
            # rows 32:128 are zeros), so leaf matmuls contract over K=128:
            # same cycle count, but 4x the PE-array power of a K=32 matmul —
            # which is what the HAM clock gate needs to grant full duty
            # (~2.4 GHz) right from the start of the stream instead of ~15us
            # into it.
            # dummy activation: pulls the ~1.3us Lrelu ACT_TABLE_LOAD to the
            # head of the kernel (overlapping the input DMAs) instead of in
            # front of the first real leaf activation
            scratch = const_pool.tile([128, 2], F32, tag="scratch")
            nc.gpsimd.memset(scratch[:], 0)
            nc.scalar.activation(scratch[:, 1:2], scratch[:, 0:1], LRELU,
                                 alpha=ALPHA)

            lv_buf = const_pool.tile([128, 3 * qs], mm_dt, tag="lvring")
            for sl in range(3):
                # partition-group rule: an access at base partition 32 may
                # span at most 32 partitions, at 64 at most 64.  The two
                # groups go to different engines so each slot's zeroing
                # (~1.8us) runs once in parallel, slot 0 first — it gates
                # the first leaf matmul.
                nc.gpsimd.memset(lv_buf[32:64, sl * qs: (sl + 1) * qs], 0)
                nc.vector.memset(lv_buf[64:128, sl * qs: (sl + 1) * qs], 0)
            started_q = [False] * n_lv_dmas

            def start_q(q):
                sl = q % 3
                nc.sync.dma_start(lv_buf[0:VAL, sl * qs: (sl + 1) * qs],
                                  lv_d[:, q * qs: (q + 1) * qs])
                started_q[q] = True

            def q_slice(q, c0, c1):
                sl = q % 3
                return lv_buf[:, sl * qs + c0: sl * qs + c1]

            start_q(0)
            nc.sync.dma_start(wp[:, 0:_WE], wp_d[:, 0:_WE])
            bias = const_pool.tile([128, 4], F32, tag="bias")
            nc.sync.dma_start(bias[:], bias_d[:])
            if n_lv_dmas > 1:
                start_q(1)



            def act_lrelu(dst_ap, src_ap, bias_col):
                nc.scalar.activation(dst_ap, src_ap, LRELU,
                                     bias=bias[:, bias_col: bias_col + 1],
                                     alpha=ALPHA)

            def dve_lrelu(dst_ap, src_ap, w):
                # dst = 0.01*x + 0.99*relu(x)  (zero-bias leaky-relu; PSUM may
                # be read only once per instruction, hence the two-op form)
                tmp = dvetmp_pool.tile([128, w], F32, tag="dvetmp", name="dvetmp")
                nc.vector.tensor_scalar(tmp[:], src_ap, 0.0, 1.0 - ALPHA,
                                        mybir.AluOpType.max, mybir.AluOpType.mult)
                nc.vector.scalar_tensor_tensor(dst_ap, src_ap, float(ALPHA), tmp[:],
                                               mybir.AluOpType.mult,
                                               mybir.AluOpType.add)

            # per-level pending buffers (binary-counter cascade)
            cur_tile = {d: None for d in range(ship_d, sub + 1)}
            cur_fill = {d: 0 for d in range(ship_d, sub + 1)}
            base_col = {d: 0 for d in range(ship_d, sub + 1)}
            consume_ctr = {"n": 0}

            def emit(d, w):
                """Reserve w columns at level d; returns (tile, offset)."""
                wd = width(d)
                if cur_tile[d] is None:
                    cur_tile[d] = pend_pool.tile([128, wd], mm_dt,
                                                 tag=f"p{d}", name=f"pend{d}")
                    cur_fill[d] = 0
                off = cur_fill[d]
                assert off + w <= wd
                cur_fill[d] = off + w
                return cur_tile[d], off

            def queue_full(d, j):
                """Detach level d's (full) pending tile and queue its consume."""
                assert cur_fill[d] == width(d)
                ready.append((d, cur_tile[d], cur_fill[d], j))
                cur_tile[d] = None
                cur_fill[d] = 0

            def _cast(m):
                # fp32r is ISA-illegal below a minimum moving size
                # (s3d3_mm_fp32r_restrictions); tiny steps fall back to fp32.
                if mm_dt != F32R or m >= f32r_min_n:
                    return lambda ap: ap
                return lambda ap: ap.bitcast(F32)

            def cell_w1(E, O, m, tail):
                """W1 matmuls + h activation for m parents; returns h_s."""
                consume_ctr["n"] += 1
                cast = _cast(m)
                w2 = 2 * m
                h = ps_h.tile([128, w2], F32, tag="h")
                nc.tensor.matmul(h[:, 0:m], cast(wp[:, 0:128]), cast(E),
                                 start=True, stop=False)
                nc.tensor.matmul(h[:, 0:m], cast(wp[:, _W1B: _W1B + 128]), cast(O),
                                 start=False, stop=True)
                nc.tensor.matmul(h[:, m:w2], cast(wp[:, 128:256]), cast(E),
                                 start=True, stop=False)
                nc.tensor.matmul(h[:, m:w2], cast(wp[:, _W1B + 128: _W1B + 256]),
                                 cast(O), start=False, stop=True)
                h_s = hs_pool.tile([128, w2], mm_dt, tag="h_s")
                if zero_bias:
                    if dve_h_every and not tail \
                            and consume_ctr["n"] % dve_h_every == 0:
                        dve_lrelu(h_s[:], h[:], w2)
                    else:
                        act_lrelu(h_s[:], h[:], 0)
                else:
                    act_lrelu(h_s[:, 0:m], h[:, 0:m], 0)
                    act_lrelu(h_s[:, m:w2], h[:, m:w2], 1)
                return h_s

            def cell_w2(h_s, m, dst, off, tail, dst_level):
                """W2 matmuls + parent-output activation into dst."""
                cast = _cast(m)
                w2 = 2 * m
                o_p = ps_o.tile([128, m], F32, tag="op")
                nc.tensor.matmul(o_p[:], cast(wp[:, _W2A: _W2A + 128]),
                                 cast(h_s[:, 0:m]), start=True, stop=False)
                nc.tensor.matmul(o_p[:], cast(wp[:, _W2B: _W2B + 128]),
                                 cast(h_s[:, m:w2]), start=False, stop=True)
                # DVE's two-op leaky-relu has ~2x the latency of ACT's native
                # one; tail tiles sit on the serial final chain of the
                # cascade, so only big mid-stream tiles go to DVE.
                if zero_bias and dve_out and m >= 512 and not tail:
                    dve_lrelu(dst[:, off: off + m], o_p[:], m)
                else:
                    act_lrelu(dst[:, off: off + m], o_p[:], 2)
                if dst_level == ship_d:
                    # The ship level is never consumed on-device: stream each
                    # slice to HBM as soon as its activation lands, so the
                    # last transfer overlaps the end of the cascade.
                    b = base_col[ship_d]
                    base_col[ship_d] = b + m
                    off0 = 2 ** ship_d - 1
                    nc.sync.dma_start(out_d[:, off0 + b: off0 + b + m],
                                      dst[:, off: off + m])

            def consume_phase1(d, t, w, tail):
                """DMA a full level-d tile out and run the W1 half of its
                parent computation (W1 matmuls + h activation)."""
                b = base_col[d]
                base_col[d] = b + w
                off0 = 2 ** d - 1
                nc.sync.dma_start(out_d[:, off0 + b: off0 + b + w], t[:, 0:w])
                if d == ship_d:
                    return None
                hw2 = w // 2
                dst, off = emit(d - 1, hw2)
                # Detach a filled pending tile immediately (so later emits to
                # this level get a fresh tile) but only queue its consume in
                # phase 2, once the instructions writing it exist.  Ship-level
                # tiles are DMA'd slice-by-slice in cell_w2, never consumed.
                det = None
                if cur_fill[d - 1] == width(d - 1):
                    if d - 1 != ship_d:
                        det = (d - 1, cur_tile[d - 1], cur_fill[d - 1])
                    cur_tile[d - 1] = None
                    cur_fill[d - 1] = 0
                parts = []
                if tail and w >= 4:
                    # Half-splitting shortens the serial latency of the final
                    # one-consume-per-level chain: half 1's matmuls overlap
                    # half 0's activations.
                    hw4 = w // 4
                    for k in (0, 1):
                        s = k * hw2
                        h_s = cell_w1(t[:, s: s + hw2: 2], t[:, s + 1: s + hw2: 2],
                                      hw4, tail)
                        parts.append((h_s, hw4, off + k * hw4))
                else:
                    parts.append((cell_w1(t[:, 0:w:2], t[:, 1:w:2], hw2, tail),
                                  hw2, off))
                return (parts, dst, det, tail, d - 1)

            def consume_phase2(st):
                parts, dst, det, tail, dst_level = st
                for h_s, m, off in parts:
                    cell_w2(h_s, m, dst, off, tail, dst_level)
                if det is not None:
                    ready.append((det[0], det[1], det[2], cur_chunk["j"]))

            # Deferred-consume queue: running a full cascade inline would put
            # a chain of dependent instructions at the head of the in-order
            # PE queue and stall it.  Instead, when a pending buffer fills it
            # is detached and queued, and consumes are drained a couple of
            # leaf chunks later — by then their inputs are old enough that
            # the PE never waits.  Consumes are drained in software-pipelined
            # pairs (phase1 = W1+h-act, phase2 = W2+out-act) so one consume's
            # W2 matmuls never sit directly behind its own h activation in
            # the in-order PE queue.
            ready = []
            cur_chunk = {"j": 0}
            carry = []    # deferred phase2 of the most recent stream consume

            def drain(n, min_age_chunk=None, tail=False):
                """Run up to n eligible consumes, software-pipelined: all
                phase1s are emitted, then the phase2 of the PREVIOUS drain's
                last consume and of all but the last of this batch.  Keeping
                one phase2 in flight across drain calls guarantees ~a chunk
                of PE work between any consume's W1 matmuls (whose results
                its h activation needs) and its W2 matmuls (which wait on
                that activation)."""
                batch = []
                for _ in range(n):
                    if not ready:
                        break
                    if min_age_chunk is not None and ready[0][3] >= min_age_chunk:
                        break
                    dd, t, f, _j = ready.pop(0)
                    batch.append((dd, t, f))
                states = [st for dd, t, f in batch
                          if (st := consume_phase1(dd, t, f, tail)) is not None]
                pend2 = carry + states
                hold = 0 if (tail or not pend2) else 1
                for st in pend2[: len(pend2) - hold]:
                    consume_phase2(st)
                carry[:] = pend2[len(pend2) - hold:]

            cur_q = 0
            for j in range(n_chunks):
                if j % chunks_per_q == 0:
                    cur_q = q = j // chunks_per_q
                    if not started_q[q]:
                        start_q(q)
                    # prefetch two DMA periods ahead so the PE never waits on
                    # a leaf-value transfer at a q boundary
                    pq = q + 2
                    if pq < n_lv_dmas and not started_q[pq]:
                        start_q(pq)
                m = j % chunks_per_q
                # chunks 1/2 borrow the (still idle) h-PSUM slots: with one
                # leaf slot, chunk 1's matmul would stall on chunk 0's
                # activation before the cascade provides cover work
                if j in (1, 2):
                    p = ps_h.tile([128, ch], F32, tag="h", name="p_early")
                else:
                    p = ps_leaf.tile([128, ch], F32, tag="pl")
                for s in range(0, ch, 512):
                    sw = min(512, ch - s)
                    nc.tensor.matmul(p[:, s: s + sw], wp[:, _WE: _WE + 128],
                                     q_slice(cur_q, m * ch + s, m * ch + s + sw),
                                     start=True, stop=True)
                dst, off = emit(sub, ch)
                if zero_bias and j % 8 == 5:
                    # keep the two elementwise engines balanced: every 8th
                    # leaf activation goes to the (lighter-loaded) DVE
                    dve_lrelu(dst[:, off: off + ch], p[:], ch)
                else:
                    act_lrelu(dst[:, off: off + ch], p[:], 3)
                if cur_fill[sub] == width(sub):
                    queue_full(sub, j)
                # drain up to 3 consumes whose inputs are at least one chunk
                # old (a consume drained right after its pending tile filled
                # stalls the PE on the producing activations; at ch=1024 one
                # chunk of lead time is ~3.4us, plenty)
                cur_chunk["j"] = j
                drain(3, min_age_chunk=j)
                # after cascade bursts, keep the backlog short so pending-
                # buffer slots recycle before the next fill needs them (but
                # never force-drain a tile finished this very chunk)
                if len(ready) > 4:
                    drain(len(ready) - 4, min_age_chunk=j)
            while ready or carry:
                drain(2, tail=True)

            assert all(cur_tile[d] is None for d in cur_tile), "unconsumed pending"
            assert all(base_col[d] == 2 ** d for d in base_col)

    # bacc passes: split multi-waits into event semaphores (HW allows one
    # sync wait per instruction), register allocation, DCE.
    nc.compile()
    return nc


def _leaky(v):
    return np.where(v >= 0, v, np.float32(ALPHA) * v).astype(np.float32)


def pack_weights(We, W1, W2):
    wpack = np.zeros((128, WPACK_COLS), np.float32)
    wpack[:, _W1A: _W1A + 256] = W1[0:128, :]
    wpack[:, _W1B: _W1B + 256] = W1[128:256, :]
    wpack[:, _W2A: _W2A + 128] = W2[0:128, :]
    wpack[:, _W2B: _W2B + 128] = W2[128:256, :]
    wpack[0:32, _WE: _WE + 128] = We
    return wpack


def pack_bias(b1, b2, be):
    bias = np.zeros((128, 4), np.float32)
    bias[:, 0] = b1[0:128]
    bias[:, 1] = b1[128:256]
    bias[:, 2] = b2
    bias[:, 3] = be
    return bias


def _np_dt(dt_):
    if dt_ == BF16:
        import ml_dtypes
        return ml_dtypes.bfloat16
    if dt_ == FP16:
        return np.float16
    return np.float32


_NC_CACHE = {}


def kernel(leaf_values, We, be, W1, b1, W2, b2, _trace=False):
    leaf_values = np.asarray(leaf_values, np.float32)
    We = np.asarray(We, np.float32)
    be = np.asarray(be, np.float32)
    W1 = np.asarray(W1, np.float32)
    b1 = np.asarray(b1, np.float32)
    W2 = np.asarray(W2, np.float32)
    b2 = np.asarray(b2, np.float32)

    sub_leaves = 2 ** SUB

    npdt = _np_dt(MM_DT)
    zero_bias = not b1.any()
    wpack = pack_weights(We, W1, W2).astype(npdt)
    bias = pack_bias(b1, b2, be)
    lvT = leaf_values.reshape(N_CORES, sub_leaves, VAL).transpose(0, 2, 1)
    in_maps = [
        {"lvT": np.ascontiguousarray(lvT[c]).astype(npdt), "wpack": wpack,
         "bias": bias}
        for c in range(N_CORES)
    ]

    key = (MM_DT, zero_bias, SHIP_D)
    if _NC_CACHE.get("key") != key:
        _NC_CACHE["nc"] = build_nc(mm_dt=MM_DT, zero_bias=zero_bias,
                                   ship_d=SHIP_D)
        _NC_CACHE["key"] = key
    nc = _NC_CACHE["nc"]

    res = run_bass_kernel_spmd(nc, in_maps, list(range(N_CORES)), trace=_trace)
    outs = [np.asarray(res.results[c]["outT"], np.float32) for c in range(N_CORES)]

    embs = np.empty((N_NODES, EMB), np.float32)
    for c in range(N_CORES):
        full = np.ascontiguousarray(outs[c].T)        # [sub_nodes, 128]
        for d in range(SHIP_D, SUB + 1):
            L = 3 + d
            n = 1 << d
            g0 = (1 << L) - 1 + c * n
            embs[g0: g0 + n] = full[n - 1: 2 * n - 1]

    # levels above the ship level on host (one batched GEMM pair per level)
    L_ship = 3 + SHIP_D
    lvl = embs[(1 << L_ship) - 1: (1 << (L_ship + 1)) - 1]
    for l in range(L_ship - 1, -1, -1):
        x = lvl.reshape(2 ** l, 2 * EMB)
        h = _leaky(x @ W1 + b1)
        lvl = _leaky(h @ W2 + b2)
        embs[(1 << l) - 1: (1 << (l + 1)) - 1] = lvl

    if _trace:
        kernel.last_results = res
    return embs

